# revision 10
# baseline (speedup 1.0000x reference)
"""Trainium2 Bass kernel for nn_BiDenseConv2d (binarized 3x3 conv + sync-BN + channel bypass).

Shapes (hardcoded): x [8, 48, 224, 224] f32 -> out [8, 64, 224, 224] f32.

Sharding: data-parallel over batch, 1 image per NeuronCore (8 cores); BN batch
stats all-reduced across cores ([64,2] f32 collective); weights replicated.

Per-core pipeline:
  1. binarize: sign(sin(2pi(x-eps)/tau)) == (frac((x-eps)/tau) < 0.5 ? +1 : -1),
     computed on DVE as mod -> is_lt -> {0,1} -> -0.5 -> {-0.5,+0.5} in fp8e4.
     Processed in a seg-major [128p] layout (partition = 16*seg + group) fed by a
     host-prearranged copy of x so every DMA is 128 partitions wide.
  2. conv: 9-tap shift-matmul, kh-pairs stacked to K=96 via a one-row-shifted
     image copy on partitions 48..95; two output blocks run concurrently via PE
     column tiling (0,0)/(0,64). fp8 +-0.5 acts x +-1 weights accumulate exact
     half-integer sums in PSUM f32; evicted to fp16 (exact).
  3. BN: sums/sumsq via accum_out on the eviction ops; AllReduce; affine
     k = gamma*s'*rsqrt(s'^2 var + eps), c = beta - mu k with s' = 2 mean|w|.
  4. bypass: identity channels stream from HBM (channel-major x input); the 16
     merge-mean channels are 3-channel group means computed in the seg-major
     layout (GPSIMD) and merged into the bypass buffer by DMA.

Conv input channel order is a permutation (slot 16c+g <-> channel 15c+g, g<15;
45+c for g=15) folded into the weights host-side.
"""
import sys
import numpy as np

sys.path.insert(0, '/opt/trn_rl_repo')

B, CIN, COUT, H, W = 8, 48, 64, 224, 224
NCORES = 8
SEGS, SEGR = 8, 28          # 8 row-segments of 28 rows
SEGQ = SEGR * W             # 6272
HSEGQ = SEGQ // 2           # 3136
NBANK = 56                  # bank b covers image rows 4b..4b+3
NEG = 14                    # eviction groups of 4 banks
PW = 226
BN_EPS = 1e-5
MAGIC = 12582912.0  # 1.5 * 2**23: fp32 round-to-int magic

_cache = {}

# slot permutation: conv channel-slot sigma = 16c+g holds channel 15c+g (g<15), 45+c (g=15)
SLOT_TO_CH = np.zeros(48, np.int64)
for _c in range(3):
    for _g in range(16):
        SLOT_TO_CH[16 * _c + _g] = (45 + _c) if _g == 15 else (15 * _c + _g)


def _build(general_affine: bool):
    from concourse import bacc, tile, mybir
    mt = mybir.dt
    AO = mybir.AluOpType
    AF = mybir.ActivationFunctionType

    nc = bacc.Bacc("TRN2", target_bir_lowering=False, debug=False,
                   num_devices=NCORES)

    # x_dev: seg-major prearranged copy (partition 16s+g, slot c, seg content)
    xdev_d = nc.dram_tensor("xdev", [128, 3, SEGQ], mt.float32, kind="ExternalInput")
    # x_chan: original channel-major image for the bypass path
    xch_d = nc.dram_tensor("xch", [CIN, H * W], mt.float32, kind="ExternalInput")
    wp_d = nc.dram_tensor("wp", [3, 96, 64], mt.float8e4, kind="ExternalInput")
    ws_d = nc.dram_tensor("ws", [3, 48, 64], mt.float8e4, kind="ExternalInput")
    cst_d = nc.dram_tensor("cst", [64, 4], mt.float32, kind="ExternalInput")
    coef_d = nc.dram_tensor("coef", [128, 8], mt.float32, kind="ExternalInput")
    out_d = nc.dram_tensor("out", [2, COUT, NBANK, 448], mt.float32,
                           kind="ExternalOutput")

    # [ch, seg, 7 bank, parity, 448] view of channel-major x for bypass loads
    xv_blk = xch_d.ap().rearrange("c (s j p w) -> c s j p w", s=SEGS, j=7, p=2)

    with tile.TileContext(nc) as tc:
        with tc.tile_pool(name="main", bufs=1) as P, \
             tc.tile_pool(name="psum", bufs=2, space="PSUM") as PS, \
             tc.tile_pool(name="dram", bufs=1, space="DRAM") as D:

            # ---- constants ----
            wp = P.tile([96, 3, 64], mt.float8e4)
            ws = P.tile([48, 3, 64], mt.float8e4)
            for kw in range(3):
                nc.sync.dma_start(wp[:, kw, :], wp_d.ap()[kw])
                nc.sync.dma_start(ws[:, kw, :], ws_d.ap()[kw])
            cst = P.tile([64, 4], mt.float32)
            nc.sync.dma_start(cst[:], cst_d.ap())
            coef = P.tile([128, 8], mt.float32)
            if general_affine:
                nc.sync.dma_start(coef[:], coef_d.ap())

            # ---- persistent tiles ----
            xa2f = P.tile([96, PW, PW], mt.float8e4)
            bm = P.tile([128, 2, HSEGQ], mt.float32)
            y = P.tile([128, NBANK, 448], mt.float16)
            sums = P.tile([128, NEG], mt.float32)
            sqs = P.tile([128, NEG], mt.float32)

            # zero borders (compute partition bases must be 0/32/64/96, so the
            # memsets span [0:96]; interior rows rewritten by scatter/B-copy)
            nc.vector.memset(xa2f[0:96, 0, :], 0.0)
            nc.vector.memset(xa2f[0:96, 224:226, :], 0.0)
            nc.vector.memset(xa2f[0:96, :, 0], 0.0)
            nc.vector.memset(xa2f[0:96, :, 225], 0.0)

            # ---- prep: load, binarize, scatter ----
            for c in range(3):
                for hf in range(2):
                    x1b = P.tile([128, HSEGQ], mt.float32, tag="x1", bufs=2,
                                 name=f"x1b_{c}_{hf}")
                    nc.sync.dma_start(
                        x1b[:], xdev_d.ap()[:, c, hf * HSEGQ:(hf + 1) * HSEGQ])
                    if general_affine:
                        nc.vector.tensor_scalar(
                            x1b[:], x1b[:], coef[:, c:c + 1], coef[:, 3 + c:4 + c],
                            AO.mult, AO.add)
                    # bypass group-sums on GPSIMD (DVE is the busy engine)
                    if c == 0:
                        nc.gpsimd.tensor_copy(bm[:, hf, :], x1b[:])
                    else:
                        nc.gpsimd.tensor_tensor(bm[:, hf, :], bm[:, hf, :],
                                                x1b[:], AO.add)
                    # sign(sin(2pi t)) = +1 iff t - rint(t) >= 0; rint via the
                    # fp32 magic constant (DVE/Pool round each op to fp32)
                    m1 = P.tile([128, HSEGQ], mt.float32, tag="xa1bp", bufs=2,
                                name=f"m1_{c}_{hf}")
                    nc.gpsimd.tensor_scalar(m1[:], x1b[:], MAGIC, None, AO.add)
                    nc.gpsimd.tensor_scalar(m1[:], m1[:], MAGIC, None,
                                            AO.subtract)
                    t2b = P.tile([128, HSEGQ], mt.bfloat16, tag="t2ob", bufs=2,
                                 name=f"t2b_{c}_{hf}")
                    nc.vector.tensor_tensor(t2b[:], x1b[:], m1[:], AO.is_ge)
                    xa1b = P.tile([128, HSEGQ], mt.float8e4, tag="xa1bp", bufs=2,
                                  name=f"xa1b_{c}_{hf}")
                    nc.vector.tensor_scalar(xa1b[:], t2b[:], 0.5, None,
                                            AO.subtract)
                    # scatter to conv layout: one 16-partition DMA per segment
                    for s in range(SEGS):
                        r0 = 1 + SEGR * s + 14 * hf
                        nc.sync.dma_start(
                            xa2f[16 * c:16 * c + 16, r0:r0 + 14, 1:225],
                            xa1b[16 * s:16 * s + 16, :].rearrange(
                                "p (r w) -> p r w", r=14))

            # B half: one-row-shifted copy of A, per segment
            for s in range(SEGS):
                nc.sync.dma_start(xa2f[48:96, SEGR * s:SEGR * s + SEGR, :],
                                  xa2f[0:48, SEGR * s + 1:SEGR * s + SEGR + 1, :])

            bmf = bm[:].rearrange("p h q -> p (h q)")
            nc.gpsimd.tensor_scalar(bmf, bmf, 1.0 / 3.0, None, AO.mult)

            # ---- conv ----
            for b4 in range(NEG):
                ps4 = PS.tile([128, 4, 512], mt.float32, tag="ps", bufs=2,
                              name=f"ps4_{b4}")
                for k in range(4):
                    b = 4 * b4 + k
                    for ci, (pb, tp) in enumerate(((0, (0, 0)), (64, (0, 64)))):
                        h0 = 4 * b + 2 * ci
                        for kw in range(3):
                            nc.tensor.matmul(
                                ps4[pb:pb + 64, k, 0:448],
                                wp[:, kw, :],
                                xa2f[0:96, h0:h0 + 2, kw:kw + 224],
                                start=(kw == 0), stop=False, tile_position=tp)
                        for kw in range(3):
                            nc.tensor.matmul(
                                ps4[pb:pb + 64, k, 0:448],
                                ws[:, kw, :],
                                xa2f[0:48, h0 + 2:h0 + 4, kw:kw + 224],
                                start=False, stop=(kw == 2), tile_position=tp)
                nc.vector.tensor_scalar(
                    y[:, 4 * b4:4 * b4 + 4, :], ps4[:, :, 0:448], 1.0, None,
                    AO.mult, AO.add, accum_out=sums[:, b4:b4 + 1])
                scr = P.tile([128, 4, 448], mt.float16, tag="scr", bufs=2,
                             name=f"scr_{b4}")
                nc.scalar.activation(scr[:], ps4[:, :, 0:448], AF.Square,
                                     accum_out=sqs[:, b4:b4 + 1])

            # ---- stats + collective ----
            ssum = P.tile([128, 1], mt.float32)
            ssq = P.tile([128, 1], mt.float32)
            nc.vector.reduce_sum(ssum[:], sums[:], axis=mybir.AxisListType.X)
            nc.vector.reduce_sum(ssq[:], sqs[:], axis=mybir.AxisListType.X)
            # walrus requires equal start partitions on compute operands, so
            # move the upper half down with a DMA before combining
            toph = P.tile([64, 2], mt.float32)
            nc.sync.dma_start(toph[:, 0:1], ssum[64:128, :])
            nc.sync.dma_start(toph[:, 1:2], ssq[64:128, :])
            cb = P.tile([64, 2], mt.float32)
            nc.vector.tensor_tensor(cb[:, 0:1], ssum[0:64, :], toph[:, 0:1], AO.add)
            nc.vector.tensor_tensor(cb[:, 1:2], ssq[0:64, :], toph[:, 1:2], AO.add)
            nc.vector.tensor_scalar(cb[:], cb[:], 1.0 / float(B * H * W), None,
                                    AO.mult)
            cbin = D.tile([64, 2], mt.float32)
            cbout = D.tile([64, 2], mt.float32)
            nc.sync.dma_start(cbin[:], cb[:])
            nc.gpsimd.collective_compute(
                "AllReduce", AO.add,
                replica_groups=[list(range(NCORES))],
                ins=[cbin.opt()], outs=[cbout.opt()])
            mv2 = P.tile([64, 2], mt.float32)
            nc.sync.dma_start(mv2[:], cbout[:])

            # k = cst1 / sqrt(var*cst0 + eps); c = cst2 - mu*k
            m2t = P.tile([64, 1], mt.float32)
            nc.vector.tensor_tensor(m2t[:], mv2[:, 0:1], mv2[:, 0:1], AO.mult)
            vart = P.tile([64, 1], mt.float32)
            nc.vector.tensor_tensor(vart[:], mv2[:, 1:2], m2t[:], AO.subtract)
            t1 = P.tile([64, 1], mt.float32)
            nc.vector.tensor_tensor(t1[:], vart[:], cst[:, 0:1], AO.mult)
            nc.vector.tensor_scalar(t1[:], t1[:], BN_EPS, None, AO.add)
            sq = P.tile([64, 1], mt.float32)
            nc.scalar.activation(sq[:], t1[:], AF.Sqrt)
            rc = P.tile([64, 1], mt.float32)
            nc.vector.reciprocal(rc[:], sq[:])
            kc = P.tile([128, 2], mt.float32)
            nc.vector.tensor_tensor(kc[0:64, 0:1], rc[:], cst[:, 1:2], AO.mult)
            mk = P.tile([64, 1], mt.float32)
            nc.vector.tensor_tensor(mk[:], mv2[:, 0:1], kc[0:64, 0:1], AO.mult)
            nc.vector.tensor_tensor(kc[0:64, 1:2], cst[:, 2:3], mk[:], AO.subtract)
            nc.sync.dma_start(kc[64:128, :], kc[0:64, :])

            # ---- pass 2: normalize + bypass + store ----
            bmflat = bm[:].rearrange("p h q -> p (h q)")
            for s in range(SEGS):
                bpb = P.tile([128, 7, 448], mt.float32, tag="xa1bp", bufs=2,
                             name=f"bpb_{s}")
                nc.sync.dma_start(bpb[0:48, :, :], xv_blk[:, s, :, 0, :])
                nc.sync.dma_start(bpb[64:112, :, :], xv_blk[:, s, :, 1, :])
                bmsrc = bmflat[16 * s:16 * s + 16, :].rearrange(
                    "p (j e) -> p j e", j=7)
                nc.sync.dma_start(bpb[48:64, :, :], bmsrc[:, :, 0:448])
                nc.sync.dma_start(bpb[112:128, :, :], bmsrc[:, :, 448:896])
                ob = P.tile([128, 7, 448], mt.float32, tag="t2ob", bufs=2,
                            name=f"ob_{s}")
                nc.scalar.activation(ob[:], y[:, 7 * s:7 * s + 7, :], AF.Identity,
                                     bias=kc[:, 1:2], scale=kc[:, 0:1])
                nc.vector.tensor_tensor(ob[:], ob[:], bpb[:], AO.add)
                # single 128-partition store: dest dims (parity, co, bank, w)
                nc.sync.dma_start(out_d.ap()[:, :, 7 * s:7 * s + 7, :], ob[:])

    nc.compile()
    return nc


def _get_nc(general_affine):
    key = ("nc", general_affine, NCORES)
    if key not in _cache:
        _cache[key] = _build(general_affine)
    return _cache[key]


def _host_prep(alpha, epsilon, tau, A, weight, gamma, beta):
    import ml_dtypes
    f8 = ml_dtypes.float8_e4m3

    eps_v = np.broadcast_to(np.asarray(epsilon, np.float32).reshape(-1), (CIN,)) \
        if np.asarray(epsilon).size in (1, CIN) else None
    eps_v = np.asarray(epsilon, np.float32).reshape(-1)
    tau_v = np.asarray(tau, np.float32).reshape(-1)
    A_v = np.asarray(A, np.float32).reshape(-1)
    if eps_v.size == 1:
        eps_v = np.full(CIN, eps_v[0], np.float32)
    if tau_v.size == 1:
        tau_v = np.full(CIN, tau_v[0], np.float32)
    if A_v.size == 1:
        A_v = np.full(CIN, A_v[0], np.float32)

    general = not (np.all(eps_v == 0.0) and np.all(tau_v == 1.0))

    w = np.asarray(weight, np.float32)
    scale = np.mean(np.abs(w), axis=(1, 2, 3), dtype=np.float32)
    sw = np.sign(w).astype(np.float32)
    waff = sw * A_v[None, :, None, None]      # fold A (exact for A=+-1 etc.)
    wperm = waff[:, SLOT_TO_CH, :, :]         # [co, slot, kh, kw]
    wp = np.ascontiguousarray(
        np.concatenate([wperm[:, :, 0, :], wperm[:, :, 1, :]], axis=1)
        .transpose(2, 1, 0)).astype(f8)       # [3, 96, 64]
    wsx = np.ascontiguousarray(wperm[:, :, 2, :].transpose(2, 1, 0)).astype(f8)

    sprime = 2.0 * scale
    cst = np.zeros((64, 4), np.float32)
    cst[:, 0] = sprime * sprime
    cst[:, 1] = np.asarray(gamma, np.float32).reshape(-1) * sprime
    cst[:, 2] = np.asarray(beta, np.float32).reshape(-1)

    coef = np.zeros((128, 8), np.float32)
    if general:
        for p in range(128):
            g = p % 16
            for c in range(3):
                ch = 45 + c if g == 15 else 15 * c + g
                coef[p, c] = 1.0 / tau_v[ch]
                coef[p, 3 + c] = -eps_v[ch] / tau_v[ch]
    return general, wp, wsx, cst, coef


def _make_xdev(xi):
    """xi [48, 224, 224] f32 -> [128, 3, 6272] seg-major layout."""
    xr = xi.reshape(CIN, SEGS, SEGQ)
    p = np.arange(128)
    s_idx = p // 16
    g_idx = p % 16
    ch = np.empty((128, 3), np.int64)
    for c in range(3):
        ch[:, c] = np.where(g_idx == 15, 45 + c, 15 * c + g_idx)
    return np.ascontiguousarray(xr[ch, s_idx[:, None], :])


def kernel(x, alpha, epsilon, tau, A, weight, gamma, beta):
    from concourse import bass_utils

    x = np.asarray(x, np.float32)
    general, wp, wsx, cst, coef = _host_prep(alpha, epsilon, tau, A,
                                             weight, gamma, beta)
    nc = _get_nc(general)

    in_maps = []
    for i in range(NCORES):
        xi = np.ascontiguousarray(x[i])
        in_maps.append({
            "xdev": _make_xdev(xi),
            "xch": xi.reshape(CIN, H * W),
            "wp": wp, "ws": wsx, "cst": cst, "coef": coef,
        })
    res = bass_utils.run_bass_kernel_spmd(nc, in_maps,
                                          core_ids=list(range(NCORES)))
    out = np.stack([
        res.results[i]["out"].reshape(2, COUT, NBANK, 2, 224)
        .transpose(1, 2, 0, 3, 4).reshape(COUT, H, W)
        for i in range(NCORES)
    ])
    return out.astype(np.float32)
